# revision 11
# baseline (speedup 1.0000x reference)
"""Trainium2 Bass kernel for nn_AttentionCrossLayer.

Math: in the reference, softmax over a length-1 axis is exactly 1.0, so
attn == v and q/k/wq/wk are dead code. With x0 the (never-mutated) input,
each layer's gate xw_i = out_i @ cw_i is a fixed linear function of x0:
    xw_i = x0 @ u_i + c_i,   u_i = Wv_i @ (Wo_i @ cw_i),
                             c_i = (bv_i @ Wo_i + bo_i) @ cw_i
and the layer recurrence x += x0 * xw_i + cb_i telescopes to
    out[b, d] = x0[b, d] * (x0[b, :] @ usum + cprime) + cbsum[d]
with usum = sum_i u_i  [D], cprime = 1 + sum_i c_i, cbsum = sum_i cb_i [D].

The tiny weight contractions happen host-side in float64. The rel-err
gate is 2e-2, so x is staged to the device in bf16 and the output is
stored in bf16 (upcast to f32 on the host): the kernel is DMA-bound and
this halves HBM traffic to 16.8MB/core. Quantization error ~0.2% RMS.

Layout: 2 consecutive x rows per SBUF partition (tile = [128, 2048]
covering 256 rows) so every DMA descriptor is a contiguous 4KB DRAM
line — at 2KB lines the 16 DMA queues hit their ~100ns/descriptor
floor instead of streaming at full HBM rate. 16 tiles stay resident.

Compute rides the Vector engine only (2x DVE rate on bf16; the Scalar
engine's activation path is 4x slower on this size and previously
tail-gated the stores): per tile, two fused multiply+row-reduce
(scalar_tensor_tensor with accum_out) give the raw per-row dots, a
[P,2] immediate-add applies cprime, and two tensor_scalar muls scale
the rows in place. Stores chase per-tile completion semaphores.

Loads issue from the sync engine (HWDGE) with a small outstanding cap;
stores issue from GpSimd (SWDGE) so the two directions ride disjoint
queue pools, with the trailing stores on the (by then idle) HWDGE via
the Scalar engine. Store completion shares ONE semaphore (only the
all-done total is awaited, so fractional multi-queue increments are
safe); loads keep per-tile semaphores (partial credit isn't).

Sharding: data-parallel over batch across 8 cores, weights replicated,
no cross-device comms.
"""

import numpy as np

L, B, D, H, K = 3, 32768, 1024, 8, 64
N_CORES = 8
B_LOC = B // N_CORES  # 4096 rows per core
P = 128
R = 2  # x rows per SBUF partition
N_TILES = B_LOC // (P * R)  # 16
FREE = R * D  # 2048 elements per partition per tile
DPP = FREE + 64  # slot stride 4224B = 128B aligned
LOAD_CAP = 6  # max outstanding sync-engine loads

_cache = {}


def _build_program(zero_cb: bool, cprime: float):
    import concourse.bass as bass
    from concourse import mybir

    F32 = mybir.dt.float32
    BF16 = mybir.dt.bfloat16
    MUL = mybir.AluOpType.mult
    ADD = mybir.AluOpType.add

    nc = bass.Bass()
    # 2 consecutive batch rows per DRAM "row" -> 4KB per partition line
    x = nc.declare_dram_parameter("x", [N_TILES * P, FREE], BF16, isOutput=False)
    u = nc.declare_dram_parameter("u", [1, D], BF16, isOutput=False)
    cb = nc.declare_dram_parameter("cb", [1, D], F32, isOutput=False)
    out = nc.declare_dram_parameter("out", [N_TILES * P, FREE], BF16, isOutput=True)

    u_bcast = bass.AP(tensor=u.ap().tensor, offset=0, ap=[[0, P], [1, D]])
    cb_bcast = bass.AP(tensor=cb.ap().tensor, offset=0, ap=[[0, P], [1, D]])

    N_SW_LOADS = 2   # leading loads on the store (SWDGE) pool
    N_SC_LOADS = 2   # leading loads on the Scalar engine's queue
    N_HW_STORES = 2  # trailing stores on the (idle by then) HWDGE pool

    with (
        nc.sbuf_tensor([P, D], BF16) as ub,
        nc.sbuf_tensor([P, D], F32) as cbb,
        nc.sbuf_tensor([P, N_TILES, DPP], BF16) as xt,
        nc.sbuf_tensor([P, 2, D], BF16) as oscr,  # throwaway STT main out
        nc.sbuf_tensor([P, N_TILES, R], F32) as tsc,   # raw row dots
        nc.sbuf_tensor([P, N_TILES, R], F32) as tsc2,  # dots + cprime
        nc.semaphore("us") as us,
        nc.semaphore("cm") as cm,  # STT accum writebacks retired
        nc.semaphore("vd") as vd,  # tiles fully scaled (per-tile +1)
        nc.semaphore("st") as st,    # SWDGE store DMAs retired (+16 each)
        nc.semaphore("st2") as st2,  # HWDGE trailing stores retired
        nc.Block() as block,
    ):
        lds = [nc.alloc_semaphore(f"ld{i}") for i in range(N_TILES)]

        @block.scalar
        def _(scalar):
            # broadcasts ride the scalar engine's DMA path so the first x
            # loads aren't queued behind them
            scalar.dma_start(out=ub[:, :], in_=u_bcast).then_inc(us, 16)
            for i in range(N_SW_LOADS, N_SW_LOADS + N_SC_LOADS):
                scalar.dma_start(
                    out=xt[:, i, 0:FREE], in_=x[i * P : (i + 1) * P, :]
                ).then_inc(lds[i], 16)
            if not zero_cb:
                scalar.dma_start(out=cbb[:, :], in_=cb_bcast).then_inc(us, 16)
            for i in range(N_TILES - N_HW_STORES, N_TILES):
                scalar.wait_ge(vd, i + 1)
                scalar.dma_start(
                    out=out[i * P : (i + 1) * P, :], in_=xt[:, i, 0:FREE]
                ).then_inc(st2, 16)
            scalar.wait_ge(st2, 16 * N_HW_STORES)

        @block.sync
        def _(sync):
            for i in range(N_SW_LOADS + N_SC_LOADS, N_TILES):
                if i >= LOAD_CAP + N_SW_LOADS + N_SC_LOADS:
                    sync.wait_ge(lds[i - LOAD_CAP], 16)
                sync.dma_start(
                    out=xt[:, i, 0:FREE], in_=x[i * P : (i + 1) * P, :]
                ).then_inc(lds[i], 16)

        @block.vector
        def _(vector):
            vector.wait_ge(us, 16 if zero_cb else 32)
            for i in range(N_TILES):
                vector.wait_ge(lds[i], 16)
                for r in range(R):
                    # oscr = x_r * u ; tsc[i,r] = sum_free = x_r . usum
                    nc.vector.scalar_tensor_tensor(
                        out=oscr[:, r, :],
                        in0=xt[:, i, r * D : (r + 1) * D],
                        scalar=1.0,
                        in1=ub[:, :],
                        op0=MUL,
                        op1=MUL,
                        accum_out=tsc[:, i, r : r + 1],
                    ).then_inc(cm, 1)
                # accum writebacks must retire before the dots are read
                vector.wait_ge(cm, (R + 1) * i + R)
                nc.vector.tensor_scalar_add(
                    out=tsc2[:, i, :], in0=tsc[:, i, :], scalar1=cprime
                ).then_inc(cm, 1)
                # ... and the add's writeback before the muls read tsc2
                # (DVE instructions don't interlock RAW across the pipe)
                vector.wait_ge(cm, (R + 1) * (i + 1))
                for r in range(R):
                    if zero_cb:
                        ins = nc.vector.tensor_scalar_mul(
                            out=xt[:, i, r * D : (r + 1) * D],
                            in0=xt[:, i, r * D : (r + 1) * D],
                            scalar1=tsc2[:, i, r : r + 1],
                        )
                    else:
                        ins = nc.vector.scalar_tensor_tensor(
                            out=xt[:, i, r * D : (r + 1) * D],
                            in0=xt[:, i, r * D : (r + 1) * D],
                            scalar=tsc2[:, i, r : r + 1],
                            in1=cbb[:, :],
                            op0=MUL,
                            op1=ADD,
                        )
                    if r == R - 1:
                        ins.then_inc(vd, 1)

        @block.gpsimd
        def _(gpsimd):
            for i in range(N_SW_LOADS):
                gpsimd.dma_start(
                    out=xt[:, i, 0:FREE], in_=x[i * P : (i + 1) * P, :]
                ).then_inc(lds[i], 16)
            for i in range(N_TILES - N_HW_STORES):
                gpsimd.wait_ge(vd, i + 1)
                gpsimd.dma_start(
                    out=out[i * P : (i + 1) * P, :], in_=xt[:, i, 0:FREE]
                ).then_inc(st, 16)
            gpsimd.wait_ge(st, 16 * (N_TILES - N_HW_STORES))

    return nc


def _precompute(wv, bv, wo, bo, cw, cb):
    """Host-side f64 contraction of the small per-layer weights."""
    usum = np.zeros(D, np.float64)
    cprime = 1.0
    for i in range(L):
        Wv = wv[i].reshape(D, H * K).astype(np.float64)
        Wo = wo[i].reshape(H * K, D).astype(np.float64)
        cwi = cw[i].reshape(D).astype(np.float64)
        wocw = Wo @ cwi
        usum += Wv @ wocw
        cprime += float(bv[i].reshape(H * K).astype(np.float64) @ wocw)
        cprime += float(bo[i].astype(np.float64) @ cwi)
    cbsum = cb.astype(np.float64).sum(axis=0)
    return usum.astype(np.float32), float(np.float32(cprime)), cbsum.astype(np.float32)


def _ensure_trace_hook_importable():
    # bass_utils unconditionally imports antenv.axon_hooks when the
    # BASS_TRACE env var is set; some images lack that module. A None
    # hook makes bass_utils skip tracing gracefully.
    try:
        import antenv.axon_hooks  # noqa: F401
    except ImportError:
        import sys
        import types

        mod = types.ModuleType("antenv.axon_hooks")
        mod.get_axon_ntff_profile_hook = lambda: None
        mod.set_axon_ntff_profile_hook = lambda hook: None
        sys.modules["antenv.axon_hooks"] = mod


def kernel(x, wq, bq, wk, bk, wv, bv, wo, bo, cw, cb):
    import ml_dtypes

    from concourse.bass_utils import run_bass_kernel_spmd

    _ensure_trace_hook_importable()

    bf16 = np.dtype(ml_dtypes.bfloat16)
    x = np.ascontiguousarray(np.asarray(x, dtype=np.float32)).astype(bf16)
    usum, cprime, cbsum = _precompute(
        np.asarray(wv), np.asarray(bv), np.asarray(wo), np.asarray(bo),
        np.asarray(cw), np.asarray(cb),
    )
    zero_cb = not np.any(cbsum)

    key = (zero_cb, cprime)
    if key not in _cache:
        _cache[key] = _build_program(zero_cb, cprime)
    nc = _cache[key]

    u2 = usum.astype(bf16).reshape(1, D)
    cb2 = cbsum.reshape(1, D)
    in_maps = [
        {
            "x": x[c * B_LOC : (c + 1) * B_LOC].reshape(N_TILES * P, FREE),
            "u": u2,
            "cb": cb2,
        }
        for c in range(N_CORES)
    ]
    res = run_bass_kernel_spmd(nc, in_maps, list(range(N_CORES)))
    out16 = np.concatenate(
        [res.results[c]["out"].reshape(B_LOC, D) for c in range(N_CORES)], axis=0
    )
    return out16.astype(np.float32)


# revision 16
# speedup vs baseline: 1.3973x; 1.3973x over previous
"""Trainium2 Bass kernel for nn_AttentionCrossLayer.

Math: in the reference, softmax over a length-1 axis is exactly 1.0, so
attn == v and q/k/wq/wk are dead code. With x0 the (never-mutated) input,
each layer's gate xw_i = out_i @ cw_i is a fixed linear function of x0:
    xw_i = x0 @ u_i + c_i,   u_i = Wv_i @ (Wo_i @ cw_i),
                             c_i = (bv_i @ Wo_i + bo_i) @ cw_i
and the layer recurrence x += x0 * xw_i + cb_i telescopes to
    out[b, d] = x0[b, d] * (x0[b, :] @ usum + cprime) + cbsum[d]
with usum = sum_i u_i  [D], cprime = 1 + sum_i c_i, cbsum = sum_i cb_i [D].

The tiny weight contractions happen host-side in float64. The rel-err
gate is 2e-2, so x is staged to the device in bf16 and the output is
stored in bf16 (upcast to f32 on the host): the kernel is DMA-bound and
this halves HBM traffic to 16.8MB/core. Quantization error ~0.2% RMS.

Layout: 2 consecutive x rows per SBUF partition (tile = [128, 2048]
covering 256 rows) so every DMA descriptor is a contiguous 4KB DRAM
line — at 2KB lines the 16 shared DMA engines pay ~2x per-descriptor
overhead and fall off the ~360GB/s aggregate rate. All 16 tiles stay
SBUF-resident. Slot layout per partition (bf16 elements):
    [64-elem pad | row0 (1024) | row1 (1024) | c1 | tail pad]
with a 1.0 constant at elements 63 and 2112. The two per-tile reduce
windows are 1025 wide ([63..1088) and [1088..2113)) so each covers its
row plus one constant element; the matching u operand is the broadcast
row [cprime, usum, cprime] read at offset 0 resp. 1. The reduce then
emits the finished gate t = x.usum + cprime with no extra add op, and
the DMA destination ([64..2112)) stays 128-byte aligned.

Engine split (measured costs): the DVE runs fused multiply+row-reduce
(scalar_tensor_tensor with accum_out) at 1 elem/lane/cycle -> 1.14us
per window, 2.3us/tile; the Scalar engine applies the gates (activation
with a per-partition f32 scale AP) at 1.16us per row-chunk, 2.3us/tile.
Both sit under the 2.9us/tile DMA pace, so the stream stays DMA-bound.
DVE instructions do NOT interlock RAW across the pipe: every cross- or
same-engine read of an accum output goes through a semaphore.

Loads all issue from the sync engine (HWDGE, 565ns each); stores from
GpSimd (SWDGE) with the trailing pair on the by-then-idle Scalar queue.
Load-done semaphores are 8, reused cyclically (load i incs lds[i%8] by
16, its consumer waits 16*(i//8+1)); issue of load i is gated on load
i-8 retiring, which doubles as the outstanding-load cap. Store
completions share one semaphore per DGE path (only the all-done total
is awaited, so fractional multi-queue increments are safe). Fewer
semaphores matter: the end-of-block teardown costs ~120ns per
allocated semaphore per engine.

Sharding: data-parallel over batch across 8 cores, weights replicated,
no cross-device comms.
"""

import numpy as np

L, B, D, H, K = 3, 32768, 1024, 8, 64
N_CORES = 8
B_LOC = B // N_CORES  # 4096 rows per core
P = 128
R = 2  # x rows per SBUF partition
N_TILES = B_LOC // (P * R)  # 16
FREE = R * D  # 2048 data elements per partition per tile
XOFF = 64  # data offset inside a slot; element 63 is the chunk-0 constant
C1 = XOFF + FREE  # element 2112 is the chunk-1 constant
DPP = 2176  # slot stride in elements; 4352B = 128B aligned
W = D + 1  # reduce window width
N_LDS = 8  # cyclic load-done semaphores; doubles as the load cap

_cache = {}


def _build_program(zero_cb: bool):
    import concourse.bass as bass
    from concourse import mybir

    F32 = mybir.dt.float32
    BF16 = mybir.dt.bfloat16
    MUL = mybir.AluOpType.mult
    ADD = mybir.AluOpType.add

    nc = bass.Bass()
    # 2 consecutive batch rows per DRAM "row" -> 4KB per partition line
    x = nc.declare_dram_parameter("x", [N_TILES * P, FREE], BF16, isOutput=False)
    u = nc.declare_dram_parameter("u", [1, D + 2], BF16, isOutput=False)
    cb = nc.declare_dram_parameter("cb", [1, D], F32, isOutput=False)
    out = nc.declare_dram_parameter("out", [N_TILES * P, FREE], BF16, isOutput=True)

    u_bcast = bass.AP(tensor=u.ap().tensor, offset=0, ap=[[0, P], [1, D + 2]])
    cb_bcast = bass.AP(tensor=cb.ap().tensor, offset=0, ap=[[0, P], [1, D]])

    N_HW_STORES = 2  # trailing stores on the (idle by then) Scalar queue
    N_SW_STORES = N_TILES - N_HW_STORES

    with (
        nc.sbuf_tensor([P, D + 2], BF16) as ub,  # [cprime, usum, cprime]
        nc.sbuf_tensor([P, D], F32) as cbb,
        nc.sbuf_tensor([P, N_TILES, DPP], BF16) as xt,
        # throwaway STT main outs; one slot per (tile, chunk) so no WAW
        # ordering is needed (the 8-deep DVE pipe would otherwise race)
        nc.sbuf_tensor([P, N_TILES, R, D + 1], BF16) as oscr,
        nc.sbuf_tensor([P, N_TILES, R], F32) as tsc,  # finished gates
        nc.semaphore("us") as us,
        nc.semaphore("cm") as cm,    # STT accum writebacks retired (DVE)
        nc.semaphore("cm2") as cm2,  # row-chunks scaled (Scalar/DVE)
        nc.semaphore("st") as st,    # SWDGE store DMAs retired (+16 each)
        nc.semaphore("st2") as st2,  # HWDGE trailing stores retired
        nc.Block() as block,
    ):
        lds = [nc.alloc_semaphore(f"ld{i}") for i in range(N_LDS)]

        def ld_target(i):
            return 16 * (i // N_LDS + 1)

        @block.sync
        def _(sync):
            for i in range(N_TILES):
                if i >= N_LDS:
                    sync.wait_ge(lds[(i - N_LDS) % N_LDS], ld_target(i - N_LDS))
                sync.dma_start(
                    out=xt[:, i, XOFF:C1], in_=x[i * P : (i + 1) * P, :]
                ).then_inc(lds[i % N_LDS], 16)

        @block.vector
        def _(vector):
            # 1.0 constants adjacent to each reduce window; they ride
            # the cm chain (DVE has no same-engine RAW interlock)
            nc.vector.memset(xt[:, :, XOFF - 1 : XOFF], 1.0).then_inc(cm, 1)
            nc.vector.memset(xt[:, :, C1 : C1 + 1], 1.0).then_inc(cm, 1)
            vector.wait_ge(us, 16 if zero_cb else 32)
            vector.wait_ge(cm, 2)
            for i in range(N_TILES):
                vector.wait_ge(lds[i % N_LDS], ld_target(i))
                for r in range(R):
                    # oscr = win * u' ; tsc[i,r] = sum = x_r . usum + cprime
                    nc.vector.scalar_tensor_tensor(
                        out=oscr[:, i, r, :],
                        in0=xt[:, i, XOFF - 1 + r * W : XOFF - 1 + (r + 1) * W],
                        scalar=1.0,
                        in1=ub[:, r : r + W],
                        op0=MUL,
                        op1=MUL,
                        accum_out=tsc[:, i, r : r + 1],
                    ).then_inc(cm, 1)
                if not zero_cb:
                    # general path: x <- x*t + cbsum on the DVE
                    vector.wait_ge(cm, 2 + R * (i + 1))
                    for r in range(R):
                        nc.vector.scalar_tensor_tensor(
                            out=xt[:, i, XOFF + r * D : XOFF + (r + 1) * D],
                            in0=xt[:, i, XOFF + r * D : XOFF + (r + 1) * D],
                            scalar=tsc[:, i, r : r + 1],
                            in1=cbb[:, :],
                            op0=MUL,
                            op1=ADD,
                        ).then_inc(cm2, 1)

        @block.scalar
        def _(scalar):
            # broadcasts ride the scalar engine's DMA path so the x loads
            # aren't queued behind them
            scalar.dma_start(out=ub[:, :], in_=u_bcast).then_inc(us, 16)
            if not zero_cb:
                scalar.dma_start(out=cbb[:, :], in_=cb_bcast).then_inc(us, 16)
            else:
                # pass 2: x_r <- x_r * t_r (cbsum == 0), per-partition
                # f32 scale AP on the activation path
                for i in range(N_TILES):
                    for r in range(R):
                        scalar.wait_ge(cm, 2 + R * i + r + 1)
                        nc.scalar.mul(
                            out=xt[:, i, XOFF + r * D : XOFF + (r + 1) * D],
                            in_=xt[:, i, XOFF + r * D : XOFF + (r + 1) * D],
                            mul=tsc[:, i, r : r + 1],
                        ).then_inc(cm2, 1)
            for i in range(N_SW_STORES, N_TILES):
                # the self-wait makes the in-place muls retire before the
                # DMA reads the tile
                scalar.wait_ge(cm2, R * (i + 1))
                scalar.dma_start(
                    out=out[i * P : (i + 1) * P, :], in_=xt[:, i, XOFF:C1]
                ).then_inc(st2, 16)
            scalar.wait_ge(st2, 16 * N_HW_STORES)

        @block.gpsimd
        def _(gpsimd):
            for i in range(N_SW_STORES):
                gpsimd.wait_ge(cm2, R * (i + 1))
                gpsimd.dma_start(
                    out=out[i * P : (i + 1) * P, :], in_=xt[:, i, XOFF:C1]
                ).then_inc(st, 16)
            gpsimd.wait_ge(st, 16 * N_SW_STORES)

    return nc


def _precompute(wv, bv, wo, bo, cw, cb):
    """Host-side f64 contraction of the small per-layer weights."""
    usum = np.zeros(D, np.float64)
    cprime = 1.0
    for i in range(L):
        Wv = wv[i].reshape(D, H * K).astype(np.float64)
        Wo = wo[i].reshape(H * K, D).astype(np.float64)
        cwi = cw[i].reshape(D).astype(np.float64)
        wocw = Wo @ cwi
        usum += Wv @ wocw
        cprime += float(bv[i].reshape(H * K).astype(np.float64) @ wocw)
        cprime += float(bo[i].astype(np.float64) @ cwi)
    cbsum = cb.astype(np.float64).sum(axis=0)
    return usum.astype(np.float32), float(np.float32(cprime)), cbsum.astype(np.float32)


def _ensure_trace_hook_importable():
    # bass_utils unconditionally imports antenv.axon_hooks when the
    # BASS_TRACE env var is set; some images lack that module. A None
    # hook makes bass_utils skip tracing gracefully.
    try:
        import antenv.axon_hooks  # noqa: F401
    except ImportError:
        import sys
        import types

        mod = types.ModuleType("antenv.axon_hooks")
        mod.get_axon_ntff_profile_hook = lambda: None
        mod.set_axon_ntff_profile_hook = lambda hook: None
        sys.modules["antenv.axon_hooks"] = mod


def kernel(x, wq, bq, wk, bk, wv, bv, wo, bo, cw, cb):
    import ml_dtypes

    from concourse.bass_utils import run_bass_kernel_spmd

    _ensure_trace_hook_importable()

    bf16 = np.dtype(ml_dtypes.bfloat16)
    x = np.ascontiguousarray(np.asarray(x, dtype=np.float32)).astype(bf16)
    usum, cprime, cbsum = _precompute(
        np.asarray(wv), np.asarray(bv), np.asarray(wo), np.asarray(bo),
        np.asarray(cw), np.asarray(cb),
    )
    zero_cb = not np.any(cbsum)

    if zero_cb not in _cache:
        _cache[zero_cb] = _build_program(zero_cb)
    nc = _cache[zero_cb]

    cp = np.float32(cprime)
    u2 = np.concatenate([[cp], usum, [cp]]).astype(bf16).reshape(1, D + 2)
    cb2 = cbsum.reshape(1, D)
    in_maps = [
        {
            "x": x[c * B_LOC : (c + 1) * B_LOC].reshape(N_TILES * P, FREE),
            "u": u2,
            "cb": cb2,
        }
        for c in range(N_CORES)
    ]
    res = run_bass_kernel_spmd(nc, in_maps, list(range(N_CORES)))
    out16 = np.concatenate(
        [res.results[c]["out"].reshape(B_LOC, D) for c in range(N_CORES)], axis=0
    )
    return out16.astype(np.float32)


# revision 22
# speedup vs baseline: 1.4329x; 1.0255x over previous
"""Trainium2 Bass kernel for nn_AttentionCrossLayer.

Math: in the reference, softmax over a length-1 axis is exactly 1.0, so
attn == v and q/k/wq/wk are dead code. With x0 the (never-mutated) input,
each layer's gate xw_i = out_i @ cw_i is a fixed linear function of x0:
    xw_i = x0 @ u_i + c_i,   u_i = Wv_i @ (Wo_i @ cw_i),
                             c_i = (bv_i @ Wo_i + bo_i) @ cw_i
and the layer recurrence x += x0 * xw_i + cb_i telescopes to
    out[b, d] = x0[b, d] * (x0[b, :] @ usum + cprime) + cbsum[d]
with usum = sum_i u_i  [D], cprime = 1 + sum_i c_i, cbsum = sum_i cb_i [D].

The tiny weight contractions happen host-side in float64. The rel-err
gate is 2e-2, so x is staged to the device in bf16 and the output is
stored in bf16 (upcast to f32 on the host): the kernel is DMA-bound and
this halves HBM traffic to 16.8MB/core. Quantization error ~0.2% RMS.

Layout: 2 consecutive x rows per SBUF partition (tile = [128, 2048]
covering 256 rows) so every DMA descriptor is a contiguous 4KB DRAM
line — at 2KB lines the 16 shared DMA engines pay ~2x per-descriptor
overhead and fall off the ~360GB/s aggregate rate. All 16 tiles stay
SBUF-resident. Slot layout per partition (bf16 elements):
    [64-elem pad | row0 (1024) | row1 (1024) | c1 | tail pad]
with a 1.0 constant at elements 63 and 2112. The two per-tile reduce
windows are 1025 wide ([63..1088) and [1088..2113)) so each covers its
row plus one constant element; the matching u operand is the broadcast
row [cprime, usum, cprime] read at offset 0 resp. 1. The reduce then
emits the finished gate t = x.usum + cprime with no extra add op, and
the DMA destination ([64..2112)) stays 128-byte aligned.

Engine split (measured costs): the DVE runs fused multiply+row-reduce
(scalar_tensor_tensor with accum_out) at 1 elem/lane/cycle -> 1.14us
per window, 2.3us/tile; the Scalar engine applies the gates (activation
with a per-partition f32 scale AP) at 1.16us per row-chunk, 2.3us/tile.
Both sit under the ~2.6us/tile DMA pace, so the stream stays DMA-bound.
DVE instructions do NOT interlock RAW across the pipe: every cross- or
same-engine read of an accum output goes through a semaphore.

Schedule lessons baked in (from perfetto traces of prior versions):
- The u broadcast issues FIRST, from the sync engine, ahead of every x
  load: the DMA engines drain roughly in issue order, so issuing it
  later parks the Vector engine ~4us behind 5MB of queued loads.
- All 16 loads issue back-to-back with no outstanding cap (everything
  is SBUF-resident; an issue cap only delays the last load and with it
  the whole tail).
- A dummy activation warms the Scalar engine's table (ACT_TABLE_LOAD,
  1.3us) off the critical path before the first real gate-multiply.
- Tile 15's pass 2 runs on the DVE (bf16 tensor_scalar at ~0.45us vs
  1.16us on Scalar) and its store issues from the DVE's own queue,
  shortening the last-tile dependency chain; Scalar covers tiles 0-14
  and issues tile 14's store; GpSimd (SWDGE) issues stores 0-13.
- No engine waits on store completion: the end-of-block all-engine
  barrier's InstDrain covers outstanding DGE transfers, and retiring
  the engines early lets the fixed ~7us end-of-block semaphore walk
  overlap the trailing store drain.

Sharding: data-parallel over batch across 8 cores, weights replicated,
no cross-device comms.
"""

import numpy as np

L, B, D, H, K = 3, 32768, 1024, 8, 64
N_CORES = 8
B_LOC = B // N_CORES  # 4096 rows per core
P = 128
R = 2  # x rows per SBUF partition
N_TILES = B_LOC // (P * R)  # 16
FREE = R * D  # 2048 data elements per partition per tile
XOFF = 64  # data offset inside a slot; element 63 is the chunk-0 constant
C1 = XOFF + FREE  # element 2112 is the chunk-1 constant
DPP = 2176  # slot stride in elements; 4352B = 128B aligned
W = D + 1  # reduce window width

_cache = {}


def _build_program(zero_cb: bool):
    import concourse.bass as bass
    from concourse import mybir

    F32 = mybir.dt.float32
    BF16 = mybir.dt.bfloat16
    MUL = mybir.AluOpType.mult
    ADD = mybir.AluOpType.add

    nc = bass.Bass()
    # 2 consecutive batch rows per DRAM "row" -> 4KB per partition line
    x = nc.declare_dram_parameter("x", [N_TILES * P, FREE], BF16, isOutput=False)
    u = nc.declare_dram_parameter("u", [1, D + 2], BF16, isOutput=False)
    cb = nc.declare_dram_parameter("cb", [1, D], F32, isOutput=False)
    out = nc.declare_dram_parameter("out", [N_TILES * P, FREE], BF16, isOutput=True)

    u_bcast = bass.AP(tensor=u.ap().tensor, offset=0, ap=[[0, P], [1, D + 2]])
    cb_bcast = bass.AP(tensor=cb.ap().tensor, offset=0, ap=[[0, P], [1, D]])

    LAST = N_TILES - 1

    with (
        nc.sbuf_tensor([P, D + 2], BF16) as ub,  # [cprime, usum, cprime]
        nc.sbuf_tensor([P, D], F32) as cbb,
        nc.sbuf_tensor([P, N_TILES, DPP], BF16) as xt,
        # throwaway STT main outs; one slot per (tile, chunk) so no WAW
        # ordering is needed (the 8-deep DVE pipe would otherwise race)
        nc.sbuf_tensor([P, N_TILES, R, D + 1], BF16) as oscr,
        nc.sbuf_tensor([P, N_TILES, R], F32) as tsc,  # finished gates
        nc.sbuf_tensor([P, 1], BF16) as warm,  # act-table warmup scratch
        nc.semaphore("us") as us,
        nc.semaphore("cm") as cm,    # STT accum writebacks retired (DVE)
        nc.semaphore("cm2") as cm2,  # row-chunks scaled (Scalar / DVE)
        nc.semaphore("cm3") as cm3,  # tile-15 chunks scaled (DVE)
        nc.semaphore("st") as st,    # SWDGE store DMAs retired
        nc.semaphore("st2") as st2,  # HWDGE store DMAs retired
        nc.Block() as block,
    ):
        lds = [nc.alloc_semaphore(f"ld{i}") for i in range(N_TILES)]

        @block.sync
        def _(sync):
            # broadcast first: queues drain in rough issue order and the
            # DVE can't start until u lands
            sync.dma_start(out=ub[:, :], in_=u_bcast).then_inc(us, 16)
            if not zero_cb:
                sync.dma_start(out=cbb[:, :], in_=cb_bcast).then_inc(us, 16)
            for i in range(N_TILES):
                sync.dma_start(
                    out=xt[:, i, XOFF:C1], in_=x[i * P : (i + 1) * P, :]
                ).then_inc(lds[i], 16)

        @block.vector
        def _(vector):
            # 1.0 constants adjacent to each reduce window; they ride
            # the cm chain (DVE has no same-engine RAW interlock)
            nc.vector.memset(xt[:, :, XOFF - 1 : XOFF], 1.0).then_inc(cm, 1)
            nc.vector.memset(xt[:, :, C1 : C1 + 1], 1.0).then_inc(cm, 1)
            vector.wait_ge(us, 16 if zero_cb else 32)
            vector.wait_ge(cm, 2)
            for i in range(N_TILES):
                vector.wait_ge(lds[i], 16)
                for r in range(R):
                    # oscr = win * u' ; tsc[i,r] = sum = x_r . usum + cprime
                    nc.vector.scalar_tensor_tensor(
                        out=oscr[:, i, r, :],
                        in0=xt[:, i, XOFF - 1 + r * W : XOFF - 1 + (r + 1) * W],
                        scalar=1.0,
                        in1=ub[:, r : r + W],
                        op0=MUL,
                        op1=MUL,
                        accum_out=tsc[:, i, r : r + 1],
                    ).then_inc(cm, 1)
                if not zero_cb:
                    # general path: x <- x*t + cbsum on the DVE
                    vector.wait_ge(cm, 2 + R * (i + 1))
                    for r in range(R):
                        nc.vector.scalar_tensor_tensor(
                            out=xt[:, i, XOFF + r * D : XOFF + (r + 1) * D],
                            in0=xt[:, i, XOFF + r * D : XOFF + (r + 1) * D],
                            scalar=tsc[:, i, r : r + 1],
                            in1=cbb[:, :],
                            op0=MUL,
                            op1=ADD,
                        ).then_inc(cm2, 1)
            if zero_cb:
                # tile 15 pass 2 + store on the DVE: shortest tail chain
                vector.wait_ge(cm, 2 + R * N_TILES)
                for r in range(R):
                    nc.vector.tensor_scalar_mul(
                        out=xt[:, LAST, XOFF + r * D : XOFF + (r + 1) * D],
                        in0=xt[:, LAST, XOFF + r * D : XOFF + (r + 1) * D],
                        scalar1=tsc[:, LAST, r : r + 1],
                    ).then_inc(cm3, 1)
                # store 15 issues from Scalar (the DVE has no DMA path);
                # cm3 orders it after the in-place muls retire

        @block.scalar
        def _(scalar):
            if zero_cb:
                # load the activation table off the critical path; read a
                # cell the u-broadcast initialized, write dead scratch
                scalar.wait_ge(us, 16)
                nc.scalar.mul(out=warm[:, :], in_=ub[:, 0:1], mul=1.0)
                # pass 2: x_r <- x_r * t_r (cbsum == 0), per-partition
                # f32 scale AP on the activation path; tiles 0..14
                for i in range(N_TILES - 1):
                    for r in range(R):
                        scalar.wait_ge(cm, 2 + R * i + r + 1)
                        nc.scalar.mul(
                            out=xt[:, i, XOFF + r * D : XOFF + (r + 1) * D],
                            in_=xt[:, i, XOFF + r * D : XOFF + (r + 1) * D],
                            mul=tsc[:, i, r : r + 1],
                        ).then_inc(cm2, 1)
                # trailing store for tile 14 on the idle HWDGE path; the
                # self-wait makes the in-place muls retire before the DMA
                scalar.wait_ge(cm2, R * (N_TILES - 1))
                scalar.dma_start(
                    out=out[(N_TILES - 2) * P : (N_TILES - 1) * P, :],
                    in_=xt[:, N_TILES - 2, XOFF:C1],
                ).then_inc(st2, 16)
                scalar.wait_ge(cm3, R)
                scalar.dma_start(
                    out=out[LAST * P :, :], in_=xt[:, LAST, XOFF:C1]
                ).then_inc(st2, 16)
                scalar.wait_ge(st2, 32)

        @block.gpsimd
        def _(gpsimd):
            n_sw = N_TILES - 2 if zero_cb else N_TILES
            for i in range(n_sw):
                gpsimd.wait_ge(cm2, R * (i + 1))
                gpsimd.dma_start(
                    out=out[i * P : (i + 1) * P, :], in_=xt[:, i, XOFF:C1]
                ).then_inc(st, 16)
            # SWDGE transfers MUST be awaited before block end: the
            # barrier's drain resets SWDGE semaphore tracking, and doing
            # so with stores in flight faults the device (observed
            # NRT_EXEC_UNIT_UNRECOVERABLE without this wait)
            gpsimd.wait_ge(st, 16 * n_sw)

    return nc


def _precompute(wv, bv, wo, bo, cw, cb):
    """Host-side f64 contraction of the small per-layer weights."""
    usum = np.zeros(D, np.float64)
    cprime = 1.0
    for i in range(L):
        Wv = wv[i].reshape(D, H * K).astype(np.float64)
        Wo = wo[i].reshape(H * K, D).astype(np.float64)
        cwi = cw[i].reshape(D).astype(np.float64)
        wocw = Wo @ cwi
        usum += Wv @ wocw
        cprime += float(bv[i].reshape(H * K).astype(np.float64) @ wocw)
        cprime += float(bo[i].astype(np.float64) @ cwi)
    cbsum = cb.astype(np.float64).sum(axis=0)
    return usum.astype(np.float32), float(np.float32(cprime)), cbsum.astype(np.float32)


def _ensure_trace_hook_importable():
    # bass_utils unconditionally imports antenv.axon_hooks when the
    # BASS_TRACE env var is set; some images lack that module. A None
    # hook makes bass_utils skip tracing gracefully.
    try:
        import antenv.axon_hooks  # noqa: F401
    except ImportError:
        import sys
        import types

        mod = types.ModuleType("antenv.axon_hooks")
        mod.get_axon_ntff_profile_hook = lambda: None
        mod.set_axon_ntff_profile_hook = lambda hook: None
        sys.modules["antenv.axon_hooks"] = mod


def kernel(x, wq, bq, wk, bk, wv, bv, wo, bo, cw, cb):
    import ml_dtypes

    from concourse.bass_utils import run_bass_kernel_spmd

    _ensure_trace_hook_importable()

    bf16 = np.dtype(ml_dtypes.bfloat16)
    x = np.ascontiguousarray(np.asarray(x, dtype=np.float32)).astype(bf16)
    usum, cprime, cbsum = _precompute(
        np.asarray(wv), np.asarray(bv), np.asarray(wo), np.asarray(bo),
        np.asarray(cw), np.asarray(cb),
    )
    zero_cb = not np.any(cbsum)

    if zero_cb not in _cache:
        _cache[zero_cb] = _build_program(zero_cb)
    nc = _cache[zero_cb]

    cp = np.float32(cprime)
    u2 = np.concatenate([[cp], usum, [cp]]).astype(bf16).reshape(1, D + 2)
    cb2 = cbsum.reshape(1, D)
    in_maps = [
        {
            "x": x[c * B_LOC : (c + 1) * B_LOC].reshape(N_TILES * P, FREE),
            "u": u2,
            "cb": cb2,
        }
        for c in range(N_CORES)
    ]
    res = run_bass_kernel_spmd(nc, in_maps, list(range(N_CORES)))
    out16 = np.concatenate(
        [res.results[c]["out"].reshape(B_LOC, D) for c in range(N_CORES)], axis=0
    )
    return out16.astype(np.float32)
